# revision 38
# baseline (speedup 1.0000x reference)
"""Trainium2 Bass kernel for HandDecoder-style GNN message passing.

Math (per batch element b):
  f = relu(MLP3([feature, coords]))                        # [N, C1=32]
  t1[i,j,h] = relu(a[j,h] + kb1[h] - a[i,h]),  a = coords @ kw1    # [N,N,8]
  t2[i,j,k] = relu(sum_h t1[i,j,h] kw2[h,k] + kb2[k])             # [N,N,16]
  g[j,k,d]  = sum_c f[j,c] kw3[k, c*16+d]                          # [N,16,16]
  out[i,d]  = relu(sum_{j,k} t2[i,j,k] g[j,k,d] + sum_c F[c] kb3[c*16+d])
  (F[c] = sum_j f[j,c])

v4 (all matmuls bf16, fp32 PSUM):
  - t1 pre-activation computed entirely on the PE: per j-chunk of 16, a
    K=8 matmul with a composite stationary [rows 0-2: -kw1 replicated;
    rows 4-7: a2b values for the chunk's 4 batches] against an
    augmented rhs [coords; one-hot batch selector]. The a2b rows are
    produced by one matmul (a2bT [32,128]) and routed into the blob via
    a 4-descriptor DRAM bounce. Evacuation = plain relu copies.
  - t2: 16 matmuls (N=512) into paired 2-bank PSUM tiles; 8 [128,1024]
    relu(+kb2) evacuations alternating DVE/ScalarE.
  - final: 64 col-tiled (128x32) matmuls, 4 concurrent PSUM quadrants;
    bias2 folded into the output relu as a per-partition bias.
  - all constant/input tensors packed into 4 DMAs.
Data-parallel over batch: 4 batch elements per core, 8 cores.
"""

import sys
import numpy as np
import ml_dtypes

for _p in ("/opt/trn_rl_repo",):
    if _p not in sys.path:
        sys.path.insert(0, _p)

import concourse.bass as bass
import concourse.bacc as bacc
import concourse.mybir as mybir
import concourse.tile as tile
from concourse.bass_utils import run_bass_kernel_spmd

B, N = 32, 128
C0, C1, C2 = 64, 32, 16
NCORES = 8
BPC = B // NCORES          # batches per core = 4
F32 = mybir.dt.float32
BF16 = mybir.dt.bfloat16
RELU = mybir.ActivationFunctionType.Relu
COPY = mybir.ActivationFunctionType.Copy
ADD = mybir.AluOpType.add
MAX = mybir.AluOpType.max
BF = ml_dtypes.bfloat16

# blobA column map [49 rows, 160 cols] — tiny critical-path DMA
A_C4X = 0          # [0:49,  0:32]
A_L49 = 32         # [0:49,  32:160]
# blobB column map [68 rows, 2896 cols]; DMA'd as several column chunks
# decode biases are folded in as extra contraction rows (ones-row trick)
B_XT = 0           # [0:68,  0:512]    row 67 = ones
B_DW1 = 512        # [0:68,  512:544]  row 67 = db1
B_DW2 = 544        # [0:33,  544:560]  row 32 = db2
B_DW3 = 560        # [0:17,  560:592]  row 16 = db3   -- chunk 1 ends at 592
B_C8T = 592        # [0:8,   592:1104] coords rows 0-2, row3=1, rows 4-7 b-sel
B_COMP = 1104      # [0:8,   1104:2128] rows 0-3 host const, rows 4-7 bounced
B_KW3 = 2128       # [0:32,  2128:2384]
B_KB3Q = 2384      # [0:32,  2384:2896] -- chunk 3: [2128, 2896)

_CACHED_NC = None


def build_nc():
    nc = bacc.Bacc("TRN2", target_bir_lowering=False, debug=False,
                   num_devices=NCORES)

    blobA_d = nc.dram_tensor("blobA", [49, 160], BF16, kind="ExternalInput").ap()
    blobB_d = nc.dram_tensor("blobB", [68, 2896], BF16, kind="ExternalInput").ap()
    kw2AB_d = nc.dram_tensor("kw2AB", [128, 256], BF16, kind="ExternalInput").ap()
    fbias_d = nc.dram_tensor("fbias", [128, 4], F32, kind="ExternalInput").ap()
    out_d = nc.dram_tensor("out", [128, N], F32, kind="ExternalOutput").ap()
    g_dram = nc.dram_tensor("gscr", [BPC * 32768], BF16).ap()
    a_dram = nc.dram_tensor("ascr", [4096], BF16).ap()
    ones_d = nc.dram_tensor("ones1", [1, BPC * N], BF16,
                            kind="ExternalInput").ap()

    with tile.TileContext(nc) as tc:
        with (
            tc.tile_pool(name="const", bufs=1) as cpool,
            tc.tile_pool(name="work", bufs=1) as wpool,
            tc.tile_pool(name="ps_misc", bufs=2, space=bass.MemorySpace.PSUM) as pmisc,
            tc.tile_pool(name="ps_t1", bufs=2, space=bass.MemorySpace.PSUM) as pt1,
            tc.tile_pool(name="ps_t2", bufs=3, space=bass.MemorySpace.PSUM) as pt2,
            tc.tile_pool(name="ps_out", bufs=1, space=bass.MemorySpace.PSUM) as pout,
        ):
            blobA = cpool.tile([49, 160], BF16, tag="blobA")
            blobB = cpool.tile([68, 2896], BF16, tag="blobB")
            kw2AB = cpool.tile([128, 256], BF16, tag="kw2AB")
            fbias = cpool.tile([128, 4], F32, tag="fbias")
            nc.scalar.dma_start(blobB[:, 0:592], blobB_d[0:68, 0:592])
            nc.sync.dma_start(blobA[:], blobA_d)
            nc.sync.dma_start(blobB[0:8, B_C8T:B_C8T + 512],
                              blobB_d[0:8, B_C8T:B_C8T + 512])
            nc.gpsimd.dma_start(blobB[0:4, B_COMP:B_COMP + 1024],
                                blobB_d[0:4, B_COMP:B_COMP + 1024])
            nc.scalar.dma_start(blobB[0:32, 2128:2896],
                                blobB_d[0:32, 2128:2896])
            nc.gpsimd.dma_start(kw2AB[:], kw2AB_d)
            nc.gpsimd.dma_start(fbias[:], fbias_d)

            c8T = blobB[0:8, B_C8T:B_C8T + 512]
            c4X = blobA[0:49, A_C4X:A_C4X + 32]
            L49 = blobA[0:49, A_L49:A_L49 + 128]
            xT = blobB[0:68, B_XT:B_XT + 512]
            dw1 = blobB[0:68, B_DW1:B_DW1 + 32]
            dw2 = blobB[0:33, B_DW2:B_DW2 + 16]
            dw3 = blobB[0:17, B_DW3:B_DW3 + 32]
            kw3p = blobB[0:32, B_KW3:B_KW3 + 256]
            kb3q = blobB[0:32, B_KB3Q:B_KB3Q + 512]
            kb2t = fbias[0:128, 3:4]

            # ---- decode MLP -> fT [32, (b,i)] bf16 ----
            # biases folded in via ones-rows; relu halves split Sc/DVE
            h1 = wpool.tile([33, BPC * N], BF16, tag="h1")
            h2 = wpool.tile([17, BPC * N], BF16, tag="h2")
            fT = wpool.tile([32, BPC * N], BF16, tag="fT")
            nc.gpsimd.dma_start(h1[32:33, :], ones_d)
            nc.gpsimd.dma_start(h2[16:17, :], ones_d)

            dec_ctr = [0]

            def dec_layer(w, x_in, out_rows, h_out):
                HB = BPC * N // 2
                for s in range(2):
                    dec_ctr[0] += 1
                    ps = pmisc.tile([out_rows, HB], F32, tag="m",
                                    name=f"dps{dec_ctr[0]}")
                    nc.tensor.matmul(ps[:], w, x_in[:, s * HB:(s + 1) * HB])
                    dst = h_out[0:out_rows, s * HB:(s + 1) * HB]
                    if s == 0:
                        nc.scalar.activation(dst, ps[:], RELU)
                    else:
                        nc.vector.tensor_scalar(dst, ps[:], 0.0, None, MAX)

            dec_layer(dw1, xT, 32, h1)

            # ---- M2': a2bT[(b,chunk), (jl,h)] = a[chunk*16+jl,h]+kb1[h]
            a2bT_ps = pmisc.tile([32, 128], F32, tag="m")
            nc.tensor.matmul(a2bT_ps[:], c4X, L49)
            a2bT = wpool.tile([32, 128], BF16, tag="a2bT")
            nc.scalar.activation(a2bT[:], a2bT_ps[:], COPY)
            # bounce into comp rows 4-7 via DRAM: dst[4+b, (c,jlh)] is
            # contiguous per b in (row-major) a_dram
            nc.sync.dma_start(a_dram, a2bT[:])
            nc.sync.dma_start(blobB[4:8, B_COMP:B_COMP + 1024],
                              a_dram.rearrange("(b x) -> b x", b=4))

            dec_layer(dw2, h1[0:33, :], 16, h2)
            dec_layer(dw3, h2[0:17, :], 32, fT)

            # ---- t1: 8 K=8 matmuls + plain relu evacuations ----
            t1_sb = [wpool.tile([128, BPC * N], BF16, tag=f"t1_{c}",
                                name=f"t1sb{c}") for c in range(8)]
            for c in range(8):
                t1_ps = pt1.tile([128, BPC * N], F32, tag="t1ps",
                                 name=f"t1ps{c}")
                nc.tensor.matmul(
                    t1_ps[:],
                    blobB[0:8, B_COMP + c * 128:B_COMP + (c + 1) * 128],
                    c8T)
                if c % 2 == 0:
                    nc.vector.tensor_scalar(t1_sb[c][:], t1_ps[:], 0.0, None,
                                            MAX)
                else:
                    nc.scalar.activation(t1_sb[c][:], t1_ps[:], RELU)

            # ---- g: g_rm[j, (b,k,d)] -> DRAM bounce -> g_all[(jl,k),(b,T,d)]
            g_rm = wpool.tile([128, BPC * 256], BF16, tag="grm")
            g_all = wpool.tile([128, BPC * 256], BF16, tag="gall")
            for half in range(2):
                g_ps = pmisc.tile([128, 512], F32, tag="m", name=f"gps{half}")
                for bb in range(2):
                    b = half * 2 + bb
                    nc.tensor.matmul(g_ps[:, bb * 256:(bb + 1) * 256],
                                     fT[0:32, b * N:(b + 1) * N], kw3p)
                dst = g_rm[:, half * 512:(half + 1) * 512]
                if half == 0:
                    nc.vector.tensor_scalar(dst, g_ps[:], 0.0, None, ADD)
                else:
                    nc.scalar.activation(dst, g_ps[:], COPY)
                for bb in range(2):
                    b = half * 2 + bb
                    qe = nc.sync if bb == 0 else nc.scalar
                    qe.dma_start(g_dram[b * 32768:(b + 1) * 32768],
                                 g_rm[:, b * 256:(b + 1) * 256])
            for b in range(BPC):
                qe = nc.sync if b % 2 == 0 else nc.scalar
                dstB = g_all[:, b * 256:(b + 1) * 256].rearrange(
                    "p (c d) -> p c d", d=16)
                srcB = g_dram[b * 32768:(b + 1) * 32768].rearrange(
                    "(c jk d) -> jk c d", jk=128, d=16)
                qe.dma_start(dstB, srcB)

            # ---- bias2 in quadrant layout: [32b+d, 1] ----
            F_f32 = wpool.tile([32, BPC], F32, tag="Ff")
            F_sb = wpool.tile([32, BPC], BF16, tag="F")
            for b in range(BPC):
                nc.vector.tensor_reduce(F_f32[:, b:b + 1],
                                        fT[0:32, b * N:(b + 1) * N],
                                        mybir.AxisListType.X, ADD)
            nc.gpsimd.tensor_copy(F_sb[:], F_f32[:])
            b2_ps = pmisc.tile([128, 1], F32, tag="m")
            for b in range(BPC):
                nc.tensor.matmul(b2_ps[:], kb3q[:, b * 128:(b + 1) * 128],
                                 F_sb[0:32, b:b + 1],
                                 start=(b == 0), stop=(b == 3))
            b2q = wpool.tile([128, 1], F32, tag="b2q")
            nc.scalar.activation(b2q[:], b2_ps[:], COPY)

            # ---- t2 (16 matmuls N=512, relu(+kb2) evacs) + finals ----
            # finals trail t2 by 2 tiles so the PE fills its psum-
            # backpressure stalls with final-contraction work
            out_ps = pout.tile([128, N], F32, tag="o")

            def final_mms(t):
                for b in range(BPC):
                    nc.tensor.matmul(
                        out_ps[32 * b:32 * b + C2, :],
                        g_all[:, b * 256 + t * 16:b * 256 + (t + 1) * 16],
                        t2_sb[t][:, b * N:(b + 1) * N],
                        start=(t == 0), stop=(t == 15),
                        tile_position=(0, 32 * b))

            t2_sb = [wpool.tile([128, BPC * N], BF16, tag=f"t2_{t}",
                                name=f"t2sb{t}") for t in range(16)]
            for t in range(16):
                c, half = divmod(t, 2)
                t2_ps = pt2.tile([128, BPC * N], F32, tag="t2ps",
                                 name=f"t2ps{t}")
                nc.tensor.matmul(t2_ps[:],
                                 kw2AB[:, half * 128:(half + 1) * 128],
                                 t1_sb[c][:])
                if t % 2 == 0:
                    nc.vector.tensor_scalar(t2_sb[t][:], t2_ps[:], kb2t, 0.0,
                                            ADD, MAX)
                else:
                    nc.scalar.activation(t2_sb[t][:], t2_ps[:], RELU,
                                         bias=kb2t)
                if t >= 2:
                    final_mms(t - 2)
            final_mms(14)
            final_mms(15)
            out_sb = wpool.tile([128, N], F32, tag="osb")
            nc.scalar.activation(out_sb[:], out_ps[:], RELU, bias=b2q[:])
            nc.sync.dma_start(out_d, out_sb[:])

    nc.compile()
    return nc


def _host_inputs(feature, coordinates_v, dw1, db1, dw2, db2, dw3, db3,
                 kw1, kb1, kw2, kb2, kw3, kb3):
    """Per-core input maps. Pure layout transforms, no FLOPs."""
    f32 = np.float32
    blobA0 = np.zeros((49, 160), f32)
    # L49: jl-selector x kw1 rows + kb1 row
    for jl in range(16):
        for c in range(3):
            blobA0[c * 16 + jl, A_L49 + jl * 8:A_L49 + (jl + 1) * 8] = kw1[c]
    blobA0[48, A_L49:A_L49 + 128] = np.tile(kb1, 16)

    blobB0 = np.zeros((68, 2896), f32)
    # comp rows 0-2: -kw1 replicated over jl (cols (c-chunk, jl, h))
    rep = np.tile((-kw1)[:, None, :], (1, 16, 1)).reshape(3, 128)
    blobB0[0:3, B_COMP:B_COMP + 1024] = np.tile(rep, (1, 8))
    blobB0[0:67, B_DW1:B_DW1 + 32] = dw1
    blobB0[67, B_DW1:B_DW1 + 32] = db1
    blobB0[0:32, B_DW2:B_DW2 + 16] = dw2
    blobB0[32, B_DW2:B_DW2 + 16] = db2
    blobB0[0:16, B_DW3:B_DW3 + 32] = dw3
    blobB0[16, B_DW3:B_DW3 + 32] = db3
    blobB0[0:32, B_KW3:B_KW3 + 256] = (
        kw3.reshape(16, 32, 16).transpose(1, 0, 2).reshape(32, 256))
    kb3r = kb3.reshape(32, 16)
    for b in range(4):
        blobB0[0:32, B_KB3Q + b * 128 + 32 * b:
               B_KB3Q + b * 128 + 32 * b + 16] = kb3r

    kw2AB = np.zeros((128, 256), f32)
    for jl8 in range(8):
        kw2AB[jl8 * 8:(jl8 + 1) * 8, jl8 * 16:(jl8 + 1) * 16] = kw2
        kw2AB[(jl8 + 8) * 8:(jl8 + 9) * 8, 128 + jl8 * 16:128 + (jl8 + 1) * 16] = kw2

    fbias = np.zeros((128, 4), f32)
    fbias[0:32, 0] = db1
    fbias[0:16, 1] = db2
    fbias[0:32, 2] = db3
    fbias[:, 3] = np.tile(kb2, 8)

    in_maps = []
    for cix in range(NCORES):
        fe = feature[cix * BPC:(cix + 1) * BPC]          # [4, 64]
        co = coordinates_v[cix * BPC:(cix + 1) * BPC]    # [4, 128, 3]
        blobA = blobA0.copy()
        blobB = blobB0.copy()
        for b in range(BPC):
            blobB[0:3, B_C8T + b * N:B_C8T + (b + 1) * N] = co[b].T
            blobB[4 + b, B_C8T + b * N:B_C8T + (b + 1) * N] = 1.0
            blobB[0:64, B_XT + b * N:B_XT + (b + 1) * N] = fe[b][:, None]
            blobB[64:67, B_XT + b * N:B_XT + (b + 1) * N] = co[b].T
        blobB[67, B_XT:B_XT + 512] = 1.0
        blobB[3, B_C8T:B_C8T + 512] = 1.0
        # c4X[(c,jl'), (b,chunk)] = co[b, chunk*16+jl', c]; row 48 = 1
        cr = co.transpose(2, 0, 1).reshape(3, BPC, 8, 16)  # [c, b, chunk, jl]
        blobA[0:48, A_C4X:A_C4X + 32] = cr.transpose(0, 3, 1, 2).reshape(48, 32)
        blobA[48, A_C4X:A_C4X + 32] = 1.0
        in_maps.append({
            "blobA": blobA.astype(BF), "blobB": blobB.astype(BF),
            "kw2AB": kw2AB.astype(BF), "fbias": fbias,
            "ones1": np.ones((1, BPC * N), BF)})
    return in_maps


def kernel(**inputs):
    global _CACHED_NC
    if _CACHED_NC is None:
        _CACHED_NC = build_nc()
    nc = _CACHED_NC
    in_maps = _host_inputs(
        np.asarray(inputs["feature"]), np.asarray(inputs["coordinates_v"]),
        np.asarray(inputs["dw1"]), np.asarray(inputs["db1"]),
        np.asarray(inputs["dw2"]), np.asarray(inputs["db2"]),
        np.asarray(inputs["dw3"]), np.asarray(inputs["db3"]),
        np.asarray(inputs["kw1"]), np.asarray(inputs["kb1"]),
        np.asarray(inputs["kw2"]), np.asarray(inputs["kb2"]),
        np.asarray(inputs["kw3"]), np.asarray(inputs["kb3"]))
    res = run_bass_kernel_spmd(nc, in_maps, list(range(NCORES)))
    out = np.empty((B, N, C2), np.float32)
    for cix in range(NCORES):
        r = res.results[cix]["out"]      # [128, N] quadrants
        for b in range(BPC):
            out[cix * BPC + b] = r[32 * b:32 * b + C2, :].T
    return out
